# revision 1
# baseline (speedup 1.0000x reference)
"""Trainium2 Bass kernel: ternary-conv BasicBlock (conv3x3 -> BN -> ReLU -> conv3x3 -> BN -> +res -> ReLU).

Sharding: data-parallel over batch across 8 NeuronCores (2 images/core),
conv weights replicated, BN batch stats exact via a tiny cross-core AllReduce.

The "noised" 1x1 extra term in the reference uses the centre tap of the same
ternary kernel; conv is linear in the weights, so it folds into the 3x3 kernel
by doubling the centre tap (done host-side during weight packing).

Conv padding: width is physically padded to 58 with zero columns so the PSUM
output stays contiguous per tap; height padding is virtual - the centre tap
runs first with start=True covering the full PSUM block and row-edge taps
accumulate into clipped row windows (PSUM has_written bits make partial-region
accumulation safe).
"""
import numpy as np
import ml_dtypes

import concourse.bass as bass
import concourse.bacc as bacc
import concourse.tile as tile
import concourse.mybir as mybir
from concourse import bass_utils

NCORES = 8
NI = 2              # images per core (batch 16 / 8 cores)
C = 256
P = 128
CT = C // P         # channel tiles of 128
H = W = 56
WP = 58             # physically padded width (zero cols 0 and 57)
BR = 8              # output rows per PSUM block
NB = H // BR        # 7 blocks per image
NTOT = 16 * H * W   # BN divisor over the full batch
BN_EPS = 1e-5

F32 = mybir.dt.float32
AF = mybir.ActivationFunctionType
ALU = mybir.AluOpType
AX = mybir.AxisListType

# matmul dtype: float32r streams at 1 cycle/row for N>=256 with ~1e-4 rel err
MM_DTYPE = "f32r"   # "f32r" | "bf16"

# rhs addressing: strided [rows, 56 of 58] vs contiguous 1-D 464-spans whose
# wrap-around lands only in the PSUM garbage columns (56, 57 of each row)
RHS_1D = False

# taps with the full-coverage centre tap first (start=True zeroes the block)
TAPS = [(1, 1)] + [(ky, kx) for ky in range(3) for kx in range(3) if (ky, kx) != (1, 1)]


def _mm_dt():
    return mybir.dt.float32r if MM_DTYPE == "f32r" else mybir.dt.bfloat16


def build(collective=True, num_devices=NCORES):
    DT = _mm_dt()
    nc = bacc.Bacc("TRN2", target_bir_lowering=False, debug=False,
                   num_devices=num_devices)
    x_d = nc.dram_tensor("x", [NI, C, H, W], F32, kind="ExternalInput")
    w_d = nc.dram_tensor("wq", [2, P, 36 * P], DT, kind="ExternalInput")
    gb_d = nc.dram_tensor("gb", [P, 8], F32, kind="ExternalInput")
    out_d = nc.dram_tensor("out", [NI, C, H, W], F32, kind="ExternalOutput")

    with tile.TileContext(nc) as tc:
        with (
            tc.tile_pool(name="const", bufs=1) as constp,
            tc.tile_pool(name="wpool", bufs=2) as wpool,
            tc.tile_pool(name="data", bufs=1) as datap,
            tc.tile_pool(name="small", bufs=1) as smallp,
            tc.tile_pool(name="psum", bufs=8, space=bass.MemorySpace.PSUM) as psump,
            tc.tile_pool(name="dram", bufs=1, space="DRAM") as dramp,
        ):
            def load_weights(conv, split=False):
                wsb = wpool.tile([P, 36, P], DT, tag="w", name=f"wsb{conv}")
                wsrc = w_d[conv].rearrange("p (g m) -> p g m", g=36)
                if split:
                    # centre-tap groups (16:20) DMA'd + rounded first: they
                    # gate the first matmul; the rest streams in behind
                    for lo, hi in ((16, 20), (0, 16), (20, 36)):
                        nc.sync.dma_start(wsb[:, lo:hi, :], wsrc[:, lo:hi, :])
                        if MM_DTYPE == "f32r":
                            nc.scalar.copy(wsb[:, lo:hi, :], wsb[:, lo:hi, :])
                else:
                    nc.sync.dma_start(wsb[:], wsrc)
                    if MM_DTYPE == "f32r":
                        # in-place rounding: the verifier requires f32r matmul
                        # operands produced by a rounding op, not a DMA
                        nc.scalar.copy(wsb[:], wsb[:])
                return wsb



            # conv1 centre-tap weight groups first: they gate the first matmul
            wsb0 = wpool.tile([P, 36, P], DT, tag="w", name="wsb0")
            wsrc0 = w_d[0].rearrange("p (g m) -> p g m", g=36)
            nc.sync.dma_start(wsb0[:, 16:20, :], wsrc0[:, 16:20, :])
            if MM_DTYPE == "f32r":
                nc.scalar.copy(wsb0[:, 16:20, :], wsb0[:, 16:20, :])

            # x staging in f32; the slots are reused later for conv1 raw outputs
            xs = [datap.tile([P, NI, H, W], F32, tag=f"raw{t}", name=f"xs{t}")
                  for t in range(CT)]
            HH = H // 2
            ROWPIECES = {0: ((0, 9), (9, HH), (HH, H)), 1: ((0, HH), (HH, H))}
            # first 9 rows of image 0 ahead of the bulk weight DMA
            for t in range(CT):
                nc.sync.dma_start(xs[t][:, 0, 0:9], x_d[0, t * P:(t + 1) * P, 0:9])
            # bulk of conv1 weights, split so each tap's groups arrive
            # just-in-time behind the centre tap (TAPS order is (0,0),(0,1),..)
            for lo, hi in ((0, 4), (4, 8), (8, 16), (20, 28), (28, 36)):
                nc.sync.dma_start(wsb0[:, lo:hi, :], wsrc0[:, lo:hi, :])
                if MM_DTYPE == "f32r":
                    nc.scalar.copy(wsb0[:, lo:hi, :], wsb0[:, lo:hi, :])

            gbsb = constp.tile([P, 8], F32)
            nc.sync.dma_start(gbsb[:], gb_d[:])
            for i in range(NI):
                for r0, r1 in ROWPIECES[i]:
                    if (i, r0) == (0, 0):
                        continue   # already issued ahead of the bulk weights
                    for t in range(CT):
                        nc.sync.dma_start(
                            xs[t][:, i, r0:r1],
                            x_d[i, t * P:(t + 1) * P, r0:r1])

            # conv inputs rounded to the matmul dtype, width-padded to 58
            xr = [datap.tile([P, NI * H * WP + 2], DT, tag=f"xr{t}",
                             name=f"xr{t}") for t in range(CT)]
            h1 = [datap.tile([P, NI * H * WP + 2], DT, tag=f"h1{t}",
                             name=f"h1_{t}") for t in range(CT)]
            def _img(buf, t, i):
                # [P, H, WP] view of image i inside the flat padded tile
                return buf[t][:, i * H * WP:(i + 1) * H * WP].rearrange(
                    "p (h w) -> p h w", h=H)
            zcol = constp.tile([P, H], F32)
            nc.vector.memset(zcol[:], 0.0)
            epsc = constp.tile([P, 1], F32)
            nc.vector.memset(epsc[:], BN_EPS)
            if RHS_1D:
                for t in range(CT):
                    for buf in (xr, h1):
                        nc.vector.tensor_copy(
                            buf[t][:, NI * H * WP:NI * H * WP + 2], zcol[:, 0:2])
            for i in range(NI):
                for t in range(CT):
                    for c in (0, WP - 1):
                        nc.vector.tensor_copy(_img(xr, t, i)[:, :, c], zcol[:])
                for r0, r1 in ROWPIECES[i]:
                    for t in range(CT):
                        nc.vector.tensor_copy(
                            _img(xr, t, i)[:, r0:r1, 1:57],
                            xs[t][:, i, r0:r1])

            def conv_pass(conv, wsb, srcs, raws):
                part_sum = [smallp.tile([P, NI * NB], F32, tag=f"ps{conv}{t}",
                                        name=f"psum{conv}{t}") for t in range(CT)]
                part_sq = [smallp.tile([P, NI * NB], F32, tag=f"pq{conv}{t}",
                                       name=f"psq{conv}{t}") for t in range(CT)]
                for i in range(NI):
                    for co in range(CT):
                        for blk in range(NB):
                            h0 = blk * BR
                            if RHS_1D:
                                pt = psump.tile([P, BR, WP], F32, tag="acc")
                            else:
                                pt = psump.tile([P, BR, W], F32, tag="acc")
                            k = 0
                            for ky, kx in TAPS:
                                # valid output rows for this tap (height pad is virtual)
                                hs = max(h0, 1 - ky)
                                he = min(h0 + BR - 1, H - ky)
                                nr = he - hs + 1
                                ri = hs + ky - 1
                                for ci in range(CT):
                                    g = ((ky * 3 + kx) * CT + ci) * CT + co
                                    if RHS_1D:
                                        st = i * H * WP + ri * WP + kx
                                        rhs = srcs[ci][:, st:st + nr * WP]
                                        outp = pt[:].rearrange(
                                            "p h w -> p (h w)")[
                                            :, (hs - h0) * WP:(hs - h0 + nr) * WP]
                                    else:
                                        rhs = _img(srcs, ci, i)[
                                            :, ri:ri + nr, kx:kx + W]
                                        outp = pt[:, hs - h0:hs - h0 + nr, :]
                                    nc.tensor.matmul(outp, wsb[:, g, :], rhs,
                                                     start=(k == 0), stop=(k == 17))
                                    k += 1
                            r = i * NB + blk
                            ptv = pt[:, :, 0:W] if RHS_1D else pt[:]
                            # evict PSUM -> SBUF f32, accumulating the channel sum
                            nc.vector.tensor_scalar(
                                raws[co][:, i, h0:h0 + BR, :],
                                ptv, 0.0, 0.0, ALU.bypass, ALU.add,
                                accum_out=part_sum[co][:, r:r + 1])
                            # channel sum-of-squares on the scalar engine,
                            # squaring the PSUM block in place (it is dead after)
                            nc.scalar.activation(ptv, ptv, AF.Square,
                                                 accum_out=part_sq[co][:, r:r + 1])
                return part_sum, part_sq

            def bn_params(conv, part_sum, part_sq):
                stats = smallp.tile([P, 4], F32, tag=f"st{conv}")
                for t in range(CT):
                    nc.vector.reduce_sum(stats[:, t:t + 1], part_sum[t][:], axis=AX.X)
                    nc.vector.reduce_sum(stats[:, 2 + t:3 + t], part_sq[t][:], axis=AX.X)
                if collective:
                    b_in = dramp.tile([P, 4], F32, tag=f"bi{conv}")
                    b_out = dramp.tile([P, 4], F32, tag=f"bo{conv}")
                    nc.gpsimd.dma_start(b_in[:], stats[:])
                    nc.gpsimd.collective_compute(
                        "AllReduce", ALU.add,
                        replica_groups=[list(range(num_devices))],
                        ins=[b_in.opt()], outs=[b_out.opt()])
                    gstats = smallp.tile([P, 4], F32, tag=f"gst{conv}")
                    nc.gpsimd.dma_start(gstats[:], b_out[:])
                else:
                    gstats = stats
                inv_n = 1.0 / NTOT
                mean = smallp.tile([P, 2], F32, tag=f"mean{conv}")
                var = smallp.tile([P, 2], F32, tag=f"var{conv}")
                m2 = smallp.tile([P, 2], F32, tag=f"m2{conv}")
                std = smallp.tile([P, 2], F32, tag=f"std{conv}")
                rstd = smallp.tile([P, 2], F32, tag=f"rstd{conv}")
                scl = smallp.tile([P, 2], F32, tag=f"scl{conv}")
                sft = smallp.tile([P, 2], F32, tag=f"sft{conv}")
                nc.vector.tensor_scalar(mean[:], gstats[:, 0:2], inv_n, None, ALU.mult)
                nc.vector.tensor_scalar(var[:], gstats[:, 2:4], inv_n, None, ALU.mult)
                nc.vector.tensor_tensor(m2[:], mean[:], mean[:], ALU.mult)
                nc.vector.tensor_tensor(var[:], var[:], m2[:], ALU.subtract)
                nc.scalar.activation(std[:], var[:], AF.Sqrt, bias=epsc[:])
                nc.vector.reciprocal(rstd[:], std[:])
                g_ap = gbsb[:, conv * 4: conv * 4 + 2]
                b_ap = gbsb[:, conv * 4 + 2: conv * 4 + 4]
                nc.vector.tensor_tensor(scl[:], g_ap, rstd[:], ALU.mult)
                nc.vector.tensor_tensor(sft[:], mean[:], scl[:], ALU.mult)
                nc.vector.tensor_tensor(sft[:], b_ap, sft[:], ALU.subtract)
                return scl, sft

            # conv1 -> BN1 -> ReLU (fused scale/shift/relu/round on ScalarE)
            raws1 = [datap.tile([P, NI, H, W], F32, tag=f"raw{t}", name=f"raws1_{t}")
                     for t in range(CT)]
            ps1, pq1 = conv_pass(0, wsb0, xr, raws1)
            wsb1 = load_weights(1)   # overlaps with BN1 stats / AllReduce
            for i in range(NI):
                for t in range(CT):
                    for c in (0, WP - 1):
                        nc.vector.tensor_copy(_img(h1, t, i)[:, :, c], zcol[:])
            scl1, sft1 = bn_params(0, ps1, pq1)
            # BN1 apply in pieces; a 9-row first piece for image 0 unlocks
            # conv2's first block (needs h1 rows 0..8) as early as possible
            for i in range(NI):
                for r0, r1 in ROWPIECES[i]:
                    for t in range(CT):
                        dst = _img(h1, t, i)[:, r0:r1, 1:57]
                        srcv = raws1[t][:, i, r0:r1]
                        nc.scalar.activation(dst, srcv, AF.Relu,
                                             bias=sft1[:, t:t + 1],
                                             scale=scl1[:, t:t + 1])

            # conv2 -> BN2; raws2 reuses the xr slots
            raws2 = [datap.tile([P, NI, H, W], F32, tag=f"xr{t}", name=f"raws2_{t}")
                     for t in range(CT)]
            ps2, pq2 = conv_pass(1, wsb1, h1, raws2)
            scl2, sft2 = bn_params(1, ps2, pq2)

            # out = relu(h1 + scl2*raw2 + sft2), computed in place in raws2.
            # quarter-image pieces; a couple of stt pieces go to GpSimd to
            # relieve the DVE, whose throughput paces this phase
            QH = H // 4
            pidx = 0
            for i in range(NI):
                for qq in range(4):
                    for t in range(CT):
                        rs = slice(qq * QH, (qq + 1) * QH)
                        v = raws2[t][:, i, rs]
                        h1v = _img(h1, t, i)[:, rs, 1:57]
                        if MM_DTYPE == "f32r":
                            h1v = h1v.bitcast(F32)
                        nc.vector.scalar_tensor_tensor(v, v, scl2[:, t:t + 1],
                                                       h1v, ALU.mult, ALU.add)
                        nc.scalar.activation(v, v, AF.Relu, bias=sft2[:, t:t + 1])
                        nc.sync.dma_start(
                            out_d[i, t * P:(t + 1) * P, rs], v)
                        pidx += 1

    nc.compile()
    return nc


def _quantize(w):
    """Ternary quantization matching reference.noised_tri_conv, on jax CPU,
    with the centre tap doubled (folds the 'noised' 1x1 einsum term)."""
    try:
        import jax
        import jax.numpy as jnp
        cpu = jax.devices("cpu")[0]
        with jax.default_device(cpu):
            wj = jnp.asarray(np.asarray(w, np.float32))
            tw = wj - jnp.mean(wj)
            mx, mn = jnp.max(tw), jnp.min(tw)
            lo = mn + (mx - mn) / 3
            hi = mx - (mx - mn) / 3
            tq = jnp.where(tw < lo, -1.0,
                           jnp.where(tw > hi, 1.0, 0.0)).astype(wj.dtype)
            tq = np.asarray(tq).copy()
    except Exception:
        wf = np.asarray(w, np.float32)
        tw = (wf - np.float32(wf.mean(dtype=np.float32))).astype(np.float32)
        mx, mn = np.float32(tw.max()), np.float32(tw.min())
        lo = np.float32(mn + (mx - mn) / np.float32(3))
        hi = np.float32(mx - (mx - mn) / np.float32(3))
        tq = np.where(tw < lo, np.float32(-1.0),
                      np.where(tw > hi, np.float32(1.0), np.float32(0.0)))
        tq = tq.astype(np.float32)
    tq[:, :, 1, 1] *= 2.0
    return tq


def _pack_weights(w1, w2):
    np_dt = np.float32 if MM_DTYPE == "f32r" else ml_dtypes.bfloat16
    wq = np.zeros((2, P, 36 * P), np_dt)
    for conv, w in enumerate((w1, w2)):
        q = _quantize(w)                      # [O=256, I=256, 3, 3]
        q6 = q.reshape(CT, P, CT, P, 3, 3)    # [co_t, pco, ci_t, pci, ky, kx]
        for ky in range(3):
            for kx in range(3):
                for ci in range(CT):
                    for co in range(CT):
                        g = ((ky * 3 + kx) * CT + ci) * CT + co
                        wq[conv, :, g * P:(g + 1) * P] = \
                            q6[co, :, ci, :, ky, kx].T.astype(np_dt)
    return wq


def _pack_gb(g1, b1, g2, b2):
    gb = np.zeros((P, 8), np.float32)
    for conv, (g, b) in enumerate(((g1, b1), (g2, b2))):
        for t in range(CT):
            gb[:, conv * 4 + t] = np.asarray(g, np.float32)[t * P:(t + 1) * P]
            gb[:, conv * 4 + 2 + t] = np.asarray(b, np.float32)[t * P:(t + 1) * P]
    return gb


_CACHE = {}


def _get_nc():
    if "nc" not in _CACHE:
        _CACHE["nc"] = build()
    return _CACHE["nc"]


def make_in_maps(x, w1, w2, g1, b1, g2, b2):
    x = np.asarray(x, np.float32)
    wq = _pack_weights(w1, w2)
    gb = _pack_gb(g1, b1, g2, b2)
    return [{"x": np.ascontiguousarray(x[NI * c: NI * (c + 1)]),
             "wq": wq, "gb": gb} for c in range(NCORES)]


def kernel(x, w1, w2, g1, b1, g2, b2):
    nc = _get_nc()
    in_maps = make_in_maps(x, w1, w2, g1, b1, g2, b2)
    res = bass_utils.run_bass_kernel_spmd(nc, in_maps, core_ids=list(range(NCORES)))
    return np.concatenate([res.results[c]["out"] for c in range(NCORES)], axis=0)



# revision 9
# speedup vs baseline: 22.1587x; 22.1587x over previous
"""Trainium2 Bass kernel: ternary-conv BasicBlock (conv3x3 -> BN -> ReLU -> conv3x3 -> BN -> +res -> ReLU).

Sharding: data-parallel over batch across 8 NeuronCores (2 images/core),
conv weights replicated, BN batch stats exact via tiny cross-core AllReduces.

The "noised" 1x1 extra term in the reference uses the centre tap of the same
ternary kernel; conv is linear in the weights, so it folds into the 3x3 kernel
by doubling the centre tap (done host-side during weight packing).

Conv padding: width is padded to 58 with zero columns HOST-SIDE, so x DMAs
straight from DRAM into the conv input buffer with fully contiguous
transfers and no staging copy; height padding is virtual - the centre tap
runs first with start=True covering the full PSUM block and row-edge taps
accumulate into clipped row windows (PSUM has_written bits make partial-region
accumulation safe).

Schedule: both convs run output-channel-tile-major.  Each tile's BN-stats
AllReduce + parameter chain launches as soon as that tile's conv finishes, so
tile 0's collective (and for conv2 its entire scale/shift/relu/store epilogue)
overlaps tile 1's matmuls; only tile 1's AllReduce + epilogue remain on the
critical tail.  A few warm-up matmuls on a zero tile run during the initial
input DMA to lift the PE HAM clock gate before real work arrives.
"""
import numpy as np
import ml_dtypes

import concourse.bass as bass
import concourse.bacc as bacc
import concourse.tile as tile
import concourse.mybir as mybir
from concourse import bass_utils

NCORES = 8
NI = 2              # images per core (batch 16 / 8 cores)
C = 256
P = 128
CT = C // P         # channel tiles of 128
H = W = 56
WP = 58             # width padded to 58 (zero cols 0 and 57), done host-side
BR = 8              # output rows per PSUM block
NB = H // BR        # 7 blocks per image
NTOT = 16 * H * W   # BN divisor over the full batch
BN_EPS = 1e-5
NWARM = 5           # HAM warm-up matmuls during the initial input DMA

F32 = mybir.dt.float32
F32R = mybir.dt.float32r
AF = mybir.ActivationFunctionType
ALU = mybir.AluOpType
AX = mybir.AxisListType

# taps with the full-coverage centre tap first (start=True zeroes the block)
TAPS = [(1, 1)] + [(ky, kx) for ky in range(3) for kx in range(3) if (ky, kx) != (1, 1)]


# weight group index: co-major so each output-channel tile's 18 groups are
# contiguous; tap order within a tile matches TAPS consumption order
def gidx(co, ky, kx, ci):
    return (co * 9 + ky * 3 + kx) * CT + ci


# BN1-apply row pieces; 9-row first piece for image 0 unlocks conv2's first
# block (needs h1 rows 0..8) as early as possible
ROWPIECES = {0: ((0, 9), (9, 28), (28, H)), 1: ((0, 28), (28, H))}


def build(collective=True, num_devices=NCORES, reps=1):
    nc = bacc.Bacc("TRN2", target_bir_lowering=False, debug=False,
                   num_devices=num_devices)
    x_d = nc.dram_tensor("x", [NI, C, H, WP], F32R, kind="ExternalInput")
    w_d = nc.dram_tensor("wq", [2, P, 36 * P], F32R, kind="ExternalInput")
    gb_d = nc.dram_tensor("gb", [P, 8], F32, kind="ExternalInput")
    out_d = nc.dram_tensor("out", [NI, C, H, W], F32, kind="ExternalOutput")

    with tile.TileContext(nc) as tc:
        with (
            tc.tile_pool(name="const", bufs=1) as constp,
            tc.tile_pool(name="wpool", bufs=2) as wpool,
            tc.tile_pool(name="data", bufs=1) as datap,
            tc.tile_pool(name="small", bufs=1) as smallp,
            tc.tile_pool(name="psum", bufs=8, space=bass.MemorySpace.PSUM) as psump,
            tc.tile_pool(name="dram", bufs=1, space="DRAM") as dramp,
        ):
            for rep in range(reps):
                zt = constp.tile([P, BR * W], F32)
                nc.vector.memset(zt[:], 0.0)
                epsc = constp.tile([P, 1], F32)
                nc.vector.memset(epsc[:], BN_EPS)

                # conv1 centre-tap groups of tile 0 first: they gate both the
                # warm-up matmuls and the first real matmul; ternary values
                # are exact in f32r so no rounding pass is needed
                wsb0 = wpool.tile([P, 36, P], F32R, tag="w", name="wsb0")
                wsrc0 = w_d[0].rearrange("p (g m) -> p g m", g=36)
                nc.sync.dma_start(wsb0[:, 8:10, :], wsrc0[:, 8:10, :])

                # HAM warm-up: zero matmuls into a scratch PSUM tile while the
                # first x rows are still in flight
                warmpt = psump.tile([P, BR * W], F32, tag="acc")
                for wi in range(NWARM):
                    nc.tensor.matmul(warmpt[:], wsb0[:, 8, :],
                                     zt[:].bitcast(F32R), start=True, stop=True)

                # conv inputs, width-padded to 58 host-side: x lands here
                # straight from DRAM, fully contiguous
                xr = [datap.tile([P, NI, H, WP], F32R, tag=f"xr{t}",
                                 name=f"xr{t}") for t in range(CT)]
                h1 = [datap.tile([P, NI, H, WP], F32R, tag=f"h1{t}",
                                 name=f"h1_{t}") for t in range(CT)]

                def _img(buf, t, i):
                    return buf[t][:, i]

                # h1 padding columns zeroed with one strided memset per tile
                # (xr's pads come in with the host-padded DMA)
                for t in range(CT):
                    nc.vector.memset(
                        h1[t].rearrange("p i h w -> p (i h) w")
                        [:, :, 0::WP - 1].bitcast(F32), 0.0)

                # x and conv1 weights interleaved on one queue in consumption
                # order: centre groups + first rows gate the first block
                for t in range(CT):
                    nc.sync.dma_start(_img(xr, t, 0)[:, 0:9],
                                      x_d[0, t * P:(t + 1) * P, 0:9])
                for lo, hi in ((0, 4), (4, 8), (10, 14), (14, 18)):
                    nc.sync.dma_start(wsb0[:, lo:hi, :], wsrc0[:, lo:hi, :])
                for r0, r1 in ((9, 32), (32, H)):
                    for t in range(CT):
                        nc.sync.dma_start(_img(xr, t, 0)[:, r0:r1],
                                          x_d[0, t * P:(t + 1) * P, r0:r1])
                for t in range(CT):
                    nc.sync.dma_start(_img(xr, t, 1)[:],
                                      x_d[1, t * P:(t + 1) * P, :])
                # conv1 tile-1 weights
                for lo, hi in ((26, 28), (18, 26), (28, 36)):
                    nc.sync.dma_start(wsb0[:, lo:hi, :], wsrc0[:, lo:hi, :])
                gbsb = constp.tile([P, 8], F32)
                nc.sync.dma_start(gbsb[:], gb_d[:])

                raws1 = [datap.tile([P, NI, H, W], F32, tag=f"raw{t}",
                                    name=f"raws1_{t}") for t in range(CT)]
                # conv2 raw outputs reuse the xr slots (xr is dead after conv1)
                raws2 = [datap.tile([P, NI, H, W], F32, tag=f"xr{t}",
                                    name=f"raws2_{t}") for t in range(CT)]

                def conv_tile(conv, co, wsb, srcs, raws, pt2):
                    """All PSUM blocks of one output-channel tile; evictions
                    accumulate per-block channel sums (pt2[:,0]) and
                    sums-of-squares (pt2[:,1])."""
                    for i in range(NI):
                        for blk in range(NB):
                            h0 = blk * BR
                            pt = psump.tile([P, BR, W], F32, tag="acc",
                                            name=f"pt{conv}_{co}_{i}_{blk}")
                            k = 0
                            for ky, kx in TAPS:
                                hs = max(h0, 1 - ky)
                                he = min(h0 + BR - 1, H - ky)
                                nr = he - hs + 1
                                ri = hs + ky - 1
                                for ci in range(CT):
                                    g = gidx(co, ky, kx, ci)
                                    rhs = _img(srcs, ci, i)[
                                        :, ri:ri + nr, kx:kx + W]
                                    nc.tensor.matmul(
                                        pt[:, hs - h0:hs - h0 + nr, :],
                                        wsb[:, g, :], rhs,
                                        start=(k == 0), stop=(k == 17))
                                    k += 1
                            r = i * NB + blk
                            nc.vector.tensor_scalar(
                                raws[co][:, i, h0:h0 + BR, :],
                                pt[:], 0.0, 0.0, ALU.bypass, ALU.add,
                                accum_out=pt2[:, 0, r:r + 1])
                            nc.scalar.activation(pt[:], pt[:], AF.Square,
                                                 accum_out=pt2[:, 1, r:r + 1])

                def stats_tile(conv, co, pt2):
                    """Tile stats -> AllReduce -> (scl, sft) BN parameters."""
                    stats = smallp.tile([P, 2], F32, tag=f"st{conv}{co}")
                    nc.vector.reduce_sum(stats[:], pt2[:], axis=AX.X)
                    if collective:
                        b_in = dramp.tile([P, 2], F32, tag=f"bi{conv}{co}")
                        b_out = dramp.tile([P, 2], F32, tag=f"bo{conv}{co}")
                        nc.gpsimd.dma_start(b_in[:], stats[:])
                        nc.gpsimd.collective_compute(
                            "AllReduce", ALU.add,
                            replica_groups=[list(range(num_devices))],
                            ins=[b_in.opt()], outs=[b_out.opt()])
                        gstats = smallp.tile([P, 2], F32, tag=f"gst{conv}{co}")
                        nc.gpsimd.dma_start(gstats[:], b_out[:])
                    else:
                        gstats = stats
                    mm = smallp.tile([P, 2], F32, tag=f"mm{conv}{co}")
                    var = smallp.tile([P, 1], F32, tag=f"var{conv}{co}")
                    std = smallp.tile([P, 1], F32, tag=f"std{conv}{co}")
                    rstd = smallp.tile([P, 1], F32, tag=f"rstd{conv}{co}")
                    scl = smallp.tile([P, 1], F32, tag=f"scl{conv}{co}")
                    sft = smallp.tile([P, 1], F32, tag=f"sft{conv}{co}")
                    # mm = [mean, E[x^2]]
                    nc.vector.tensor_scalar(mm[:], gstats[:], 1.0 / NTOT,
                                            None, ALU.mult)
                    mean = mm[:, 0:1]
                    nc.vector.tensor_tensor(var[:], mean, mean, ALU.mult)
                    nc.vector.tensor_tensor(var[:], mm[:, 1:2], var[:],
                                            ALU.subtract)
                    nc.scalar.activation(std[:], var[:], AF.Sqrt, bias=epsc[:])
                    nc.vector.reciprocal(rstd[:], std[:])
                    g_ap = gbsb[:, conv * 4 + co: conv * 4 + co + 1]
                    b_ap = gbsb[:, conv * 4 + 2 + co: conv * 4 + 3 + co]
                    nc.vector.tensor_tensor(scl[:], g_ap, rstd[:], ALU.mult)
                    nc.vector.tensor_tensor(sft[:], mean, scl[:], ALU.mult)
                    nc.vector.tensor_tensor(sft[:], b_ap, sft[:], ALU.subtract)
                    return scl, sft

                def apply_bn1(co, scl, sft):
                    # h1[co] = relu(scl*raw1 + sft), rounded to f32r
                    for i in range(NI):
                        for r0, r1 in ROWPIECES[i]:
                            nc.scalar.activation(
                                _img(h1, co, i)[:, r0:r1, 1:57],
                                raws1[co][:, i, r0:r1], AF.Relu,
                                bias=sft[:], scale=scl[:])

                def epilogue(co, scl, sft):
                    # out = relu(h1 + scl*raw2 + sft): 7-row compute pieces
                    # for fast pipeline start, 14-row DMA pieces to halve the
                    # descriptor cost on the SP queue
                    QH = 7
                    for i in range(NI):
                        for q in range(H // QH):
                            rs = slice(q * QH, (q + 1) * QH)
                            v = raws2[co][:, i, rs]
                            h1v = _img(h1, co, i)[:, rs, 1:57].bitcast(F32)
                            nc.vector.scalar_tensor_tensor(v, v, scl[:], h1v,
                                                           ALU.mult, ALU.add)
                            nc.scalar.activation(v, v, AF.Relu, bias=sft[:])
                            if q % 2 == 1:
                                ds = slice((q - 1) * QH, (q + 1) * QH)
                                nc.sync.dma_start(
                                    out_d[i, co * P:(co + 1) * P, ds],
                                    raws2[co][:, i, ds])

                pt2_1 = [smallp.tile([P, 2, NI * NB], F32, tag=f"pp1{co}",
                                     name=f"pt2_1_{co}") for co in range(CT)]
                pt2_2 = [smallp.tile([P, 2, NI * NB], F32, tag=f"pp2{co}",
                                     name=f"pt2_2_{co}") for co in range(CT)]

                # ---- conv1, tile-major; tile-0 stats/AllReduce overlap tile 1
                conv_tile(0, 0, wsb0, xr, raws1, pt2_1[0])
                scl10, sft10 = stats_tile(0, 0, pt2_1[0])
                # conv2 weights stream in during conv1's second tile
                wsb1 = wpool.tile([P, 36, P], F32R, tag="w", name="wsb1")
                wsrc1 = w_d[1].rearrange("p (g m) -> p g m", g=36)
                for lo, hi in ((0, 18), (18, 36)):
                    nc.sync.dma_start(wsb1[:, lo:hi, :], wsrc1[:, lo:hi, :])
                conv_tile(0, 1, wsb0, xr, raws1, pt2_1[1])
                scl11, sft11 = stats_tile(0, 1, pt2_1[1])
                apply_bn1(0, scl10, sft10)
                apply_bn1(1, scl11, sft11)

                # ---- conv2, tile-major; tile-0 epilogue overlaps tile 1 ----
                conv_tile(1, 0, wsb1, h1, raws2, pt2_2[0])
                scl20, sft20 = stats_tile(1, 0, pt2_2[0])
                conv_tile(1, 1, wsb1, h1, raws2, pt2_2[1])
                scl21, sft21 = stats_tile(1, 1, pt2_2[1])
                epilogue(0, scl20, sft20)
                epilogue(1, scl21, sft21)

    nc.compile()
    return nc


def _quantize(w):
    """Ternary quantization matching reference.noised_tri_conv, on jax CPU,
    with the centre tap doubled (folds the 'noised' 1x1 einsum term)."""
    try:
        import jax
        import jax.numpy as jnp
        cpu = jax.devices("cpu")[0]
        with jax.default_device(cpu):
            wj = jnp.asarray(np.asarray(w, np.float32))
            tw = wj - jnp.mean(wj)
            mx, mn = jnp.max(tw), jnp.min(tw)
            lo = mn + (mx - mn) / 3
            hi = mx - (mx - mn) / 3
            tq = jnp.where(tw < lo, -1.0,
                           jnp.where(tw > hi, 1.0, 0.0)).astype(wj.dtype)
            tq = np.asarray(tq).copy()
    except Exception:
        wf = np.asarray(w, np.float32)
        tw = (wf - np.float32(wf.mean(dtype=np.float32))).astype(np.float32)
        mx, mn = np.float32(tw.max()), np.float32(tw.min())
        lo = np.float32(mn + (mx - mn) / np.float32(3))
        hi = np.float32(mx - (mx - mn) / np.float32(3))
        tq = np.where(tw < lo, np.float32(-1.0),
                      np.where(tw > hi, np.float32(1.0), np.float32(0.0)))
        tq = tq.astype(np.float32)
    tq[:, :, 1, 1] *= 2.0
    return tq


def _pack_weights(w1, w2):
    wq = np.zeros((2, P, 36 * P), np.float32)
    for conv, w in enumerate((w1, w2)):
        q = _quantize(w)                      # [O=256, I=256, 3, 3]
        q6 = q.reshape(CT, P, CT, P, 3, 3)    # [co_t, pco, ci_t, pci, ky, kx]
        for ky in range(3):
            for kx in range(3):
                for ci in range(CT):
                    for co in range(CT):
                        g = gidx(co, ky, kx, ci)
                        wq[conv, :, g * P:(g + 1) * P] = \
                            q6[co, :, ci, :, ky, kx].T.astype(np.float32)
    return wq


def _pack_gb(g1, b1, g2, b2):
    gb = np.zeros((P, 8), np.float32)
    for conv, (g, b) in enumerate(((g1, b1), (g2, b2))):
        for t in range(CT):
            gb[:, conv * 4 + t] = np.asarray(g, np.float32)[t * P:(t + 1) * P]
            gb[:, conv * 4 + 2 + t] = np.asarray(b, np.float32)[t * P:(t + 1) * P]
    return gb


_CACHE = {}


def _get_nc():
    if "nc" not in _CACHE:
        _CACHE["nc"] = build()
    return _CACHE["nc"]


def make_in_maps(x, w1, w2, g1, b1, g2, b2):
    x = np.asarray(x, np.float32)
    xp = np.zeros((x.shape[0], C, H, WP), np.float32)
    xp[:, :, :, 1:57] = x
    wq = _pack_weights(w1, w2)
    gb = _pack_gb(g1, b1, g2, b2)
    return [{"x": np.ascontiguousarray(xp[NI * c: NI * (c + 1)]),
             "wq": wq, "gb": gb} for c in range(NCORES)]


def kernel(x, w1, w2, g1, b1, g2, b2):
    nc = _get_nc()
    in_maps = make_in_maps(x, w1, w2, g1, b1, g2, b2)
    res = bass_utils.run_bass_kernel_spmd(nc, in_maps, core_ids=list(range(NCORES)))
    return np.concatenate([res.results[c]["out"] for c in range(NCORES)], axis=0)


# revision 10
# speedup vs baseline: 30.0235x; 1.3549x over previous
"""Trainium2 Bass kernel: ternary-conv BasicBlock (conv3x3 -> BN -> ReLU -> conv3x3 -> BN -> +res -> ReLU).

Sharding: data-parallel over batch across 8 NeuronCores (2 images/core),
conv weights replicated, BN batch stats exact via tiny cross-core AllReduces.

The "noised" 1x1 extra term in the reference uses the centre tap of the same
ternary kernel; conv is linear in the weights, so it folds into the 3x3 kernel
by doubling the centre tap (done host-side during weight packing).

Conv padding: width is padded to 58 with zero columns HOST-SIDE, so x DMAs
straight from DRAM into the conv input buffer with fully contiguous
transfers and no staging copy; height padding is virtual - the centre tap
runs first with start=True covering the full PSUM block and row-edge taps
accumulate into clipped row windows (PSUM has_written bits make partial-region
accumulation safe).

Schedule: both convs run output-channel-tile-major.  Each tile's BN-stats
AllReduce + parameter chain launches as soon as that tile's conv finishes, so
tile 0's collective (and for conv2 its entire scale/shift/relu/store epilogue)
overlaps tile 1's matmuls; only tile 1's AllReduce + epilogue remain on the
critical tail.  A few warm-up matmuls on a zero tile run during the initial
input DMA to lift the PE HAM clock gate before real work arrives.
"""
import numpy as np
import ml_dtypes

import concourse.bass as bass
import concourse.bacc as bacc
import concourse.tile as tile
import concourse.mybir as mybir
from concourse import bass_utils

NCORES = 8
NI = 2              # images per core (batch 16 / 8 cores)
C = 256
P = 128
CT = C // P         # channel tiles of 128
H = W = 56
WP = 58             # width padded to 58 (zero cols 0 and 57), done host-side
BR = 8              # output rows per PSUM block
NB = H // BR        # 7 blocks per image
NTOT = 16 * H * W   # BN divisor over the full batch
BN_EPS = 1e-5
NWARM = 5           # HAM warm-up matmuls during the initial input DMA

F32 = mybir.dt.float32
F32R = mybir.dt.float32r
AF = mybir.ActivationFunctionType
ALU = mybir.AluOpType
AX = mybir.AxisListType

# taps with the full-coverage centre tap first (start=True zeroes the block)
TAPS = [(1, 1)] + [(ky, kx) for ky in range(3) for kx in range(3) if (ky, kx) != (1, 1)]


# weight group index: co-major so each output-channel tile's 18 groups are
# contiguous; tap order within a tile matches TAPS consumption order
def gidx(co, ky, kx, ci):
    return (co * 9 + ky * 3 + kx) * CT + ci


# BN1-apply row pieces; 9-row first piece for image 0 unlocks conv2's first
# block (needs h1 rows 0..8) as early as possible
ROWPIECES = {0: ((0, 9), (9, 28), (28, H)), 1: ((0, 28), (28, H))}


def build(collective=True, num_devices=NCORES, reps=1):
    nc = bacc.Bacc("TRN2", target_bir_lowering=False, debug=False,
                   num_devices=num_devices)
    x_d = nc.dram_tensor("x", [NI, C, H, WP], F32R, kind="ExternalInput")
    w_d = nc.dram_tensor("wq", [2, P, 36 * P], F32R, kind="ExternalInput")
    gb_d = nc.dram_tensor("gb", [P, 8], F32, kind="ExternalInput")
    out_d = nc.dram_tensor("out", [NI, C, H, W], F32, kind="ExternalOutput")

    with tile.TileContext(nc) as tc:
        with (
            tc.tile_pool(name="const", bufs=1) as constp,
            tc.tile_pool(name="wpool", bufs=2) as wpool,
            tc.tile_pool(name="data", bufs=1) as datap,
            tc.tile_pool(name="small", bufs=1) as smallp,
            tc.tile_pool(name="psum", bufs=8, space=bass.MemorySpace.PSUM) as psump,
            tc.tile_pool(name="dram", bufs=1, space="DRAM") as dramp,
        ):
            for rep in range(reps):
                zt = constp.tile([P, BR * W], F32)
                nc.vector.memset(zt[:], 0.0)
                epsc = constp.tile([P, 1], F32)
                nc.vector.memset(epsc[:], BN_EPS)
                # per-column scale [-1/N, +1/N] for the fused stats chain
                cscale = constp.tile([P, 2], F32)
                nc.vector.memset(cscale[:, 0:1], -1.0 / NTOT)
                nc.vector.memset(cscale[:, 1:2], 1.0 / NTOT)

                wsb0 = wpool.tile([P, 36, P], F32R, tag="w", name="wsb0")
                wsrc0 = w_d[0].rearrange("p (g m) -> p g m", g=36)

                # HAM warm-up: zero matmuls into a scratch PSUM tile while the
                # first x rows are still in flight
                warmpt = psump.tile([P, BR * W], F32, tag="acc")
                for wi in range(NWARM):
                    nc.tensor.matmul(warmpt[:], wsb0[:, 8, :],
                                     zt[:].bitcast(F32R), start=True, stop=True)

                # conv inputs, width-padded to 58 host-side: x lands here
                # straight from DRAM, fully contiguous
                xr = [datap.tile([P, NI, H, WP], F32R, tag=f"xr{t}",
                                 name=f"xr{t}") for t in range(CT)]
                h1 = [datap.tile([P, NI, H, WP], F32R, tag=f"h1{t}",
                                 name=f"h1_{t}") for t in range(CT)]

                def _img(buf, t, i):
                    return buf[t][:, i]

                # h1 padding columns zeroed with one strided memset per tile
                # (xr's pads come in with the host-padded DMA)
                for t in range(CT):
                    nc.vector.memset(
                        h1[t].rearrange("p i h w -> p (i h) w")
                        [:, :, 0::WP - 1].bitcast(F32), 0.0)

                # x and conv1 weights interleaved on one queue in consumption
                # order: the first x rows + centre-tap groups gate the first
                # real matmul (warm-ups wait on the centre groups too)
                for t in range(CT):
                    nc.sync.dma_start(_img(xr, t, 0)[:, 0:9],
                                      x_d[0, t * P:(t + 1) * P, 0:9])
                nc.sync.dma_start(wsb0[:, 8:10, :], wsrc0[:, 8:10, :])
                for lo, hi in ((0, 4), (4, 8), (10, 14), (14, 18)):
                    nc.sync.dma_start(wsb0[:, lo:hi, :], wsrc0[:, lo:hi, :])
                for r0, r1 in ((9, 32), (32, H)):
                    for t in range(CT):
                        nc.sync.dma_start(_img(xr, t, 0)[:, r0:r1],
                                          x_d[0, t * P:(t + 1) * P, r0:r1])
                for t in range(CT):
                    nc.sync.dma_start(_img(xr, t, 1)[:],
                                      x_d[1, t * P:(t + 1) * P, :])
                # conv1 tile-1 weights
                for lo, hi in ((26, 28), (18, 26), (28, 36)):
                    nc.sync.dma_start(wsb0[:, lo:hi, :], wsrc0[:, lo:hi, :])
                gbsb = constp.tile([P, 8], F32)
                nc.sync.dma_start(gbsb[:], gb_d[:])

                raws1 = [datap.tile([P, NI, H, W], F32, tag=f"raw{t}",
                                    name=f"raws1_{t}") for t in range(CT)]
                # conv2 raw outputs reuse the xr slots (xr is dead after conv1)
                raws2 = [datap.tile([P, NI, H, W], F32, tag=f"xr{t}",
                                    name=f"raws2_{t}") for t in range(CT)]

                def conv_tile(conv, co, wsb, srcs, raws, pt2):
                    """All PSUM blocks of one output-channel tile; evictions
                    accumulate per-block channel sums (pt2[:,0]) and
                    sums-of-squares (pt2[:,1])."""
                    for i in range(NI):
                        for blk in range(NB):
                            h0 = blk * BR
                            pt = psump.tile([P, BR, W], F32, tag="acc",
                                            name=f"pt{conv}_{co}_{i}_{blk}")
                            k = 0
                            for ky, kx in TAPS:
                                hs = max(h0, 1 - ky)
                                he = min(h0 + BR - 1, H - ky)
                                nr = he - hs + 1
                                ri = hs + ky - 1
                                for ci in range(CT):
                                    g = gidx(co, ky, kx, ci)
                                    rhs = _img(srcs, ci, i)[
                                        :, ri:ri + nr, kx:kx + W]
                                    nc.tensor.matmul(
                                        pt[:, hs - h0:hs - h0 + nr, :],
                                        wsb[:, g, :], rhs,
                                        start=(k == 0), stop=(k == 17))
                                    k += 1
                            r = i * NB + blk
                            nc.vector.tensor_scalar(
                                raws[co][:, i, h0:h0 + BR, :],
                                pt[:], 0.0, 0.0, ALU.bypass, ALU.add,
                                accum_out=pt2[:, 0, r:r + 1])
                            nc.scalar.activation(pt[:], pt[:], AF.Square,
                                                 accum_out=pt2[:, 1, r:r + 1])

                def stats_tile(conv, co, pt2):
                    """Tile stats -> AllReduce -> (scl, sft) BN parameters."""
                    stats = smallp.tile([P, 2], F32, tag=f"st{conv}{co}")
                    nc.vector.reduce_sum(stats[:], pt2[:], axis=AX.X)
                    if collective:
                        b_in = dramp.tile([P, 2], F32, tag=f"bi{conv}{co}")
                        b_out = dramp.tile([P, 2], F32, tag=f"bo{conv}{co}")
                        nc.gpsimd.dma_start(b_in[:], stats[:])
                        nc.gpsimd.collective_compute(
                            "AllReduce", ALU.add,
                            replica_groups=[list(range(num_devices))],
                            ins=[b_in.opt()], outs=[b_out.opt()])
                        gstats = smallp.tile([P, 2], F32, tag=f"gst{conv}{co}")
                        nc.gpsimd.dma_start(gstats[:], b_out[:])
                    else:
                        gstats = stats
                    mm = smallp.tile([P, 2], F32, tag=f"mm{conv}{co}")
                    nvar = smallp.tile([P, 1], F32, tag=f"var{conv}{co}")
                    std = smallp.tile([P, 1], F32, tag=f"std{conv}{co}")
                    rstd = smallp.tile([P, 1], F32, tag=f"rstd{conv}{co}")
                    scl = smallp.tile([P, 1], F32, tag=f"scl{conv}{co}")
                    sft = smallp.tile([P, 1], F32, tag=f"sft{conv}{co}")
                    # mm = [-mean, E[x^2]]; the negated mean lets the var and
                    # shift steps fuse into single scalar_tensor_tensor ops
                    nc.vector.tensor_tensor(mm[:], gstats[:], cscale[:],
                                            ALU.mult)
                    nmean = mm[:, 0:1]
                    # nvar = mean^2 - E[x^2] = -var
                    nc.vector.scalar_tensor_tensor(nvar[:], nmean, nmean,
                                                   mm[:, 1:2], ALU.mult,
                                                   ALU.subtract)
                    nc.scalar.activation(std[:], nvar[:], AF.Sqrt,
                                         bias=epsc[:], scale=-1.0)
                    nc.vector.reciprocal(rstd[:], std[:])
                    g_ap = gbsb[:, conv * 4 + co: conv * 4 + co + 1]
                    b_ap = gbsb[:, conv * 4 + 2 + co: conv * 4 + 3 + co]
                    nc.vector.tensor_tensor(scl[:], g_ap, rstd[:], ALU.mult)
                    # sft = (-mean)*scl + beta
                    nc.vector.scalar_tensor_tensor(sft[:], nmean, scl[:],
                                                   b_ap, ALU.mult, ALU.add)
                    return scl, sft

                def apply_bn1(co, scl, sft):
                    # h1[co] = relu(scl*raw1 + sft), rounded to f32r
                    for i in range(NI):
                        for r0, r1 in ROWPIECES[i]:
                            nc.scalar.activation(
                                _img(h1, co, i)[:, r0:r1, 1:57],
                                raws1[co][:, i, r0:r1], AF.Relu,
                                bias=sft[:], scale=scl[:])

                def epilogue(co, scl, sft):
                    # out = relu(h1 + scl*raw2 + sft): 7-row compute pieces
                    # for fast pipeline start, 14-row DMA pieces to halve the
                    # descriptor cost on the SP queue
                    QH = 7
                    for i in range(NI):
                        for q in range(H // QH):
                            rs = slice(q * QH, (q + 1) * QH)
                            v = raws2[co][:, i, rs]
                            h1v = _img(h1, co, i)[:, rs, 1:57].bitcast(F32)
                            nc.vector.scalar_tensor_tensor(v, v, scl[:], h1v,
                                                           ALU.mult, ALU.add)
                            nc.scalar.activation(v, v, AF.Relu, bias=sft[:])
                            if q % 2 == 1:
                                ds = slice((q - 1) * QH, (q + 1) * QH)
                                deng = nc.scalar if q % 4 == 3 else nc.sync
                                deng.dma_start(
                                    out_d[i, co * P:(co + 1) * P, ds],
                                    raws2[co][:, i, ds])

                pt2_1 = [smallp.tile([P, 2, NI * NB], F32, tag=f"pp1{co}",
                                     name=f"pt2_1_{co}") for co in range(CT)]
                pt2_2 = [smallp.tile([P, 2, NI * NB], F32, tag=f"pp2{co}",
                                     name=f"pt2_2_{co}") for co in range(CT)]

                # ---- conv1, tile-major; tile-0 stats/AllReduce overlap tile 1
                conv_tile(0, 0, wsb0, xr, raws1, pt2_1[0])
                scl10, sft10 = stats_tile(0, 0, pt2_1[0])
                # conv2 weights stream in during conv1's second tile
                wsb1 = wpool.tile([P, 36, P], F32R, tag="w", name="wsb1")
                wsrc1 = w_d[1].rearrange("p (g m) -> p g m", g=36)
                for lo, hi in ((0, 18), (18, 36)):
                    nc.sync.dma_start(wsb1[:, lo:hi, :], wsrc1[:, lo:hi, :])
                conv_tile(0, 1, wsb0, xr, raws1, pt2_1[1])
                scl11, sft11 = stats_tile(0, 1, pt2_1[1])
                apply_bn1(0, scl10, sft10)
                apply_bn1(1, scl11, sft11)

                # ---- conv2, tile-major; tile-0 epilogue overlaps tile 1 ----
                conv_tile(1, 0, wsb1, h1, raws2, pt2_2[0])
                scl20, sft20 = stats_tile(1, 0, pt2_2[0])
                conv_tile(1, 1, wsb1, h1, raws2, pt2_2[1])
                scl21, sft21 = stats_tile(1, 1, pt2_2[1])
                epilogue(0, scl20, sft20)
                epilogue(1, scl21, sft21)

    nc.compile()
    return nc


def _quantize(w):
    """Ternary quantization matching reference.noised_tri_conv, on jax CPU,
    with the centre tap doubled (folds the 'noised' 1x1 einsum term)."""
    try:
        import jax
        import jax.numpy as jnp
        cpu = jax.devices("cpu")[0]
        with jax.default_device(cpu):
            wj = jnp.asarray(np.asarray(w, np.float32))
            tw = wj - jnp.mean(wj)
            mx, mn = jnp.max(tw), jnp.min(tw)
            lo = mn + (mx - mn) / 3
            hi = mx - (mx - mn) / 3
            tq = jnp.where(tw < lo, -1.0,
                           jnp.where(tw > hi, 1.0, 0.0)).astype(wj.dtype)
            tq = np.asarray(tq).copy()
    except Exception:
        wf = np.asarray(w, np.float32)
        tw = (wf - np.float32(wf.mean(dtype=np.float32))).astype(np.float32)
        mx, mn = np.float32(tw.max()), np.float32(tw.min())
        lo = np.float32(mn + (mx - mn) / np.float32(3))
        hi = np.float32(mx - (mx - mn) / np.float32(3))
        tq = np.where(tw < lo, np.float32(-1.0),
                      np.where(tw > hi, np.float32(1.0), np.float32(0.0)))
        tq = tq.astype(np.float32)
    tq[:, :, 1, 1] *= 2.0
    return tq


def _pack_weights(w1, w2):
    wq = np.zeros((2, P, 36 * P), np.float32)
    for conv, w in enumerate((w1, w2)):
        q = _quantize(w)                      # [O=256, I=256, 3, 3]
        q6 = q.reshape(CT, P, CT, P, 3, 3)    # [co_t, pco, ci_t, pci, ky, kx]
        for ky in range(3):
            for kx in range(3):
                for ci in range(CT):
                    for co in range(CT):
                        g = gidx(co, ky, kx, ci)
                        wq[conv, :, g * P:(g + 1) * P] = \
                            q6[co, :, ci, :, ky, kx].T.astype(np.float32)
    return wq


def _pack_gb(g1, b1, g2, b2):
    gb = np.zeros((P, 8), np.float32)
    for conv, (g, b) in enumerate(((g1, b1), (g2, b2))):
        for t in range(CT):
            gb[:, conv * 4 + t] = np.asarray(g, np.float32)[t * P:(t + 1) * P]
            gb[:, conv * 4 + 2 + t] = np.asarray(b, np.float32)[t * P:(t + 1) * P]
    return gb


_CACHE = {}


def _get_nc():
    if "nc" not in _CACHE:
        _CACHE["nc"] = build()
    return _CACHE["nc"]


def make_in_maps(x, w1, w2, g1, b1, g2, b2):
    x = np.asarray(x, np.float32)
    xp = np.zeros((x.shape[0], C, H, WP), np.float32)
    xp[:, :, :, 1:57] = x
    wq = _pack_weights(w1, w2)
    gb = _pack_gb(g1, b1, g2, b2)
    return [{"x": np.ascontiguousarray(xp[NI * c: NI * (c + 1)]),
             "wq": wq, "gb": gb} for c in range(NCORES)]


def kernel(x, w1, w2, g1, b1, g2, b2):
    nc = _get_nc()
    in_maps = make_in_maps(x, w1, w2, g1, b1, g2, b2)
    res = bass_utils.run_bass_kernel_spmd(nc, in_maps, core_ids=list(range(NCORES)))
    return np.concatenate([res.results[c]["out"] for c in range(NCORES)], axis=0)
